# revision 1
# baseline (speedup 1.0000x reference)
"""Trainium2 Bass kernel for nn_BHSDuelingDQN (gnn_message_passing).

Math notes (validated vs reference to fp32 precision):
  - The edge MLP input is ones(E,1), so every edge shares one theta [F,OUT]:
        theta = (relu(w1[0]+b1) @ w2 + b2).reshape(F, OUT)
  - edge_index values live in [0, N), so the gather/scatter-add only touches
    batch 0 of flat=[B*N,F].  With C[t,s] = #edges(src=s, tgt=t):
        agg(batch0) = C @ (x[0] @ theta)
    which turns the whole message passing into dense matmuls.

Sharding: phase 1 is node-sharded (each of 8 cores owns 32 of 256 nodes and
computes partial pre-activations of adv/v1 for all 1024 batches over its
4096 feature rows).  Phase 2 is batch-sharded (each core sums the 8 partials
for its 128 batches and runs the small value-head + dueling combine).  The
host only slices / concatenates / transposes arrays between phases.

Layouts are transposed ([feature, batch]) throughout so no on-device
transposes are ever needed.
"""

import os
from contextlib import ExitStack

import numpy as np

import concourse.bacc as bacc
import concourse.bass as bass
import concourse.mybir as mybir
import concourse.tile as tile
from concourse import masks
from concourse.bass_utils import run_bass_kernel_spmd

F32 = mybir.dt.float32

B, N, F, E, OUT, NDIV, PER = 1024, 256, 8, 1024, 128, 64, 3
NADV = NDIV * PER            # 192
AV = NADV + 64               # 256 fused output cols of phase 1 (adv | v1)
M = 8                        # cores
NPC = N // M                 # 32 nodes per core
KPC = NPC * OUT // 128       # 32 K-tiles of 128 per core

# hot-path matmul dtype mode: 'f32' (exact, 4x slower PE), 'f32r' (full rate,
# reduced-precision fp32 path), 'bf16' (half DMA traffic)
MODE = os.environ.get("BASS_KERNEL_MODE", "f32r")
# nodes per conv unit (1, 2, or 4): >1 exploits PE 32-row-group concurrency
CONV_U = int(os.environ.get("BASS_CONV_U", "1"))

_build_cache = {}


def _np_store_dtype(mode):
    if mode == "bf16":
        import ml_dtypes

        return np.dtype(ml_dtypes.bfloat16)
    return np.dtype(np.float32)


def _store_dt(mode):
    if mode == "bf16":
        return mybir.dt.bfloat16
    if mode == "f32r":
        return mybir.dt.float32r  # fp32 bytes, PE full-rate reduced-precision
    return F32


PA_W1T, PA_B1T, PA_CB, PA_B2T, PA_X0T, PA_W2, PA_RW4 = 0, 1, 2, 3, 11, 267, 1291
PA_X0TL = 1419
PA_COLS = 1419 + NPC
P2_RW4, P2_BROW, P2_ONES = 0, OUT, OUT + AV
P2_COLS = OUT + AV + 512
PB_COLS = 8 * N + 8 * NPC  # soh | tohl


def _build_phase1(mode, repeat=1):
    store_dt = _store_dt(mode)
    nc = bacc.Bacc("TRN2")

    pa_d = nc.dram_tensor("pa", [128, PA_COLS], F32, kind="ExternalInput")
    pa2_d = nc.dram_tensor("pa2", [128, P2_COLS], store_dt, kind="ExternalInput")
    pb_d = nc.dram_tensor("pb", [128, PB_COLS], F32, kind="ExternalInput")
    xw_d = nc.dram_tensor("xw", [128, 8, 2048], store_dt, kind="ExternalInput")
    pt_d = nc.dram_tensor("pt", [AV, B], F32, kind="ExternalOutput")

    with tile.TileContext(nc) as tc:
      for rep in range(repeat):
        with ExitStack() as ctx:
            const = ctx.enter_context(tc.tile_pool(name=f"const{rep}", bufs=1))

            # one small-pack DMA on the scalar HWDGE queue; the big
            # streaming loads go on the sync queue in consumption order
            pa2_sb = const.tile([128, P2_COLS], store_dt)
            nc.scalar.dma_start(out=pa2_sb, in_=pa2_d[:])
            pa_sb = const.tile([128, PA_COLS], F32)
            nc.scalar.dma_start(out=pa_sb, in_=pa_d[:])
            xw_sb = const.tile([128, 8, 2048], store_dt)
            pb_sb = const.tile([128, PB_COLS], F32)
            # first chunk split so the first conv's x arrives sooner
            nc.sync.dma_start(out=xw_sb[:, 0, 0:1024], in_=xw_d[:, 0, 0:1024])
            nc.sync.dma_start(out=xw_sb[:, 0, 1024:2048], in_=xw_d[:, 0, 1024:2048])
            for g in range(1, 8):
                nc.sync.dma_start(out=xw_sb[:, g, :], in_=xw_d[:, g, :])
                if g == 1:
                    nc.sync.dma_start(out=pb_sb, in_=pb_d[:])

            def pa(col, ncols, nrows=128):
                return pa_sb[0:nrows, col : col + ncols]

            rw4_sb = pa2_sb[:, P2_RW4 : P2_RW4 + OUT]
            cb_ap = pa(PA_CB, 1)
            # head-bias row (nonzero on core 0 only) + ones row: injected
            # into the accumulators as K=1 matmuls so phase 2 needs no bias
            brow_sb = pa2_sb[0:1, P2_BROW : P2_BROW + AV]
            ones_sb = pa2_sb[0:1, P2_ONES : P2_ONES + 512]

            ident_sb = const.tile([128, 128], F32)
            masks.make_identity(nc, ident_sb)

            h_sb = const.tile([64, 1], F32)
            thT_sb = const.tile([128, F], F32)
            th_sb = const.tile([F, OUT], F32)
            x0th_sb = const.tile([128, 2, OUT], F32)
            ctl_sb = const.tile([128, 2, NPC], F32)
            feat0_sb = const.tile([128, NPC], store_dt)

            # --- phase 0a: theta and x0@theta (exact fp32) --------------
            with tc.tile_pool(name=f"p0a{rep}", bufs=1, space="PSUM") as p0a:
                # h = relu(w1 + b1) on DVE (no activation-table dep)
                nc.vector.tensor_scalar(
                    h_sb,
                    pa(PA_W1T, 1, 64),
                    pa(PA_B1T, 1, 64),
                    0.0,
                    mybir.AluOpType.add,
                    mybir.AluOpType.max,
                )
                # thetaT[oc, f] = w2[:, f*128+oc]^T @ h   (8 N=1 matmuls)
                thT_ps = p0a.tile([128, F], F32, tag="p0")
                for f in range(F):
                    nc.tensor.matmul(
                        thT_ps[:, f : f + 1],
                        pa(PA_W2 + f * OUT, OUT, 64),
                        h_sb,
                    )
                nc.vector.tensor_add(thT_sb, thT_ps, pa(PA_B2T, F))
                # theta [f, oc] via PE transpose
                th_ps = p0a.tile([F, OUT], F32, tag="p0")
                nc.tensor.transpose(th_ps, thT_sb[:, 0:F], ident_sb)
                nc.vector.tensor_copy(th_sb, th_ps)
                # x0 @ theta  [256 src, 128 oc]
                for s in range(2):
                    x0th_ps = p0a.tile(
                        [128, OUT], F32, name=f"x0th_ps{s}", tag="p0"
                    )
                    nc.tensor.matmul(
                        x0th_ps, pa(PA_X0T + s * 128, 128, F), th_sb
                    )
                    nc.vector.tensor_copy(x0th_sb[:, s, :], x0th_ps)

            # --- main pools --------------------------------------------
            acc_pool = ctx.enter_context(
                tc.tile_pool(name=f"accp{rep}", bufs=1, space="PSUM")
            )
            conv_pool = ctx.enter_context(
                tc.tile_pool(name=f"convp{rep}", bufs={1: 4, 2: 2, 4: 1}[CONV_U], space="PSUM")
            )
            feat_pool = ctx.enter_context(tc.tile_pool(name=f"featp{rep}", bufs=4))
            out_pool = ctx.enter_context(tc.tile_pool(name=f"outp{rep}", bufs=1))
            acc_ps = [acc_pool.tile([128, B], F32, name=f"acc{m}") for m in range(2)]
            out_sb0 = out_pool.tile([128, B], F32, name="out_sb0")
            out_sb1 = out_pool.tile([128, B], F32, name="out_sb1")

            # Software-pipelined sweeps (see docstring): h=1 sweep first,
            # it does not depend on the agg chain.
            pending = None

            def wc_ap(n, m):
                return xw_sb[
                    :, n // 4, 1024 + (n % 4) * 256 + m * 128 : 1024 + (n % 4) * 256 + (m + 1) * 128
                ]

            U = CONV_U

            def emit_unit(n0, h, parity):
                # one unit = U consecutive nodes on distinct 32-partition
                # row groups (the PE overlaps their K=8 convs) x half batch
                nonlocal pending
                conv_ps = conv_pool.tile([128, U, 512], F32, name="conv_ps")
                for i in range(U):
                    j = (n0 + i) % 4
                    nc.tensor.matmul(
                        conv_ps[:, i, :],
                        rw4_sb[32 * j : 32 * j + F, :],
                        xw_sb[
                            32 * j : 32 * j + F,
                            (n0 + i) // 4,
                            h * 512 : (h + 1) * 512,
                        ],
                        tile_position=(32 * j, 0),
                    )
                feat_sb = feat_pool.tile([128, U, 512], store_dt, name="feat_sb")
                for i in range(U):
                    if (i + parity) % 2 == 0:
                        nc.scalar.activation(
                            feat_sb[:, i, :],
                            conv_ps[:, i, :],
                            mybir.ActivationFunctionType.Relu,
                            bias=cb_ap,
                        )
                    else:
                        nc.vector.tensor_scalar(
                            feat_sb[:, i, :],
                            conv_ps[:, i, :],
                            cb_ap,
                            0.0,
                            mybir.AluOpType.add,
                            mybir.AluOpType.max,
                        )
                if h == 0:
                    # batch 0 lives in column 0: overwrite the nodes'
                    # first columns with the precomputed feat0 values
                    nc.vector.tensor_copy(
                        feat_sb[:, :, 0:1],
                        feat0_sb[:, n0 : n0 + U].unsqueeze(2),
                    )
                flush_acc()
                pending = (n0, h, feat_sb)

            def bias_mm(h):
                # acc[m][:, h-half] = brow^T @ ones  (writes the bias row
                # into every accumulator column; start of the PSUM group)
                for m in range(2):
                    nc.tensor.matmul(
                        acc_ps[m][:, h * 512 : (h + 1) * 512],
                        brow_sb[:, m * 128 : (m + 1) * 128],
                        ones_sb,
                        start=True,
                        stop=False,
                    )

            def flush_acc():
                nonlocal pending
                if pending is not None:
                    pn0, ph, pf = pending
                    for i in range(U):
                        n = pn0 + i
                        for m in range(2):
                            nc.tensor.matmul(
                                acc_ps[m][:, ph * 512 : (ph + 1) * 512],
                                wc_ap(n, m),
                                pf[:, i, :],
                                start=False,
                                stop=(n == NPC - 1),
                            )
                    pending = None

            def drain_half(h):
                sl = slice(h * 512, (h + 1) * 512)
                nc.vector.tensor_copy(out_sb0[:, sl], acc_ps[0][:, sl])
                nc.scalar.activation(
                    out_sb1[:, sl],
                    acc_ps[1][:, sl],
                    mybir.ActivationFunctionType.Copy,
                )
                nc.sync.dma_start(out=pt_d[0:128, sl], in_=out_sb0[:, sl])
                nc.scalar.dma_start(out=pt_d[128:AV, sl], in_=out_sb1[:, sl])

            bias_mm(1)
            for u, n0 in enumerate(range(0, NPC, U)):
                emit_unit(n0, 1, u % 2)
            flush_acc()
            drain_half(1)

            # --- phase 0b: local count matrix and agg (needs pb) --------
            if True:
                p0b = conv_pool
                for s in range(2):
                    ctl_ps = p0b.tile(
                        [128, NPC], F32, name=f"ctl_ps{s}", tag="conv_ps"
                    )
                    for k in range(8):
                        nc.tensor.matmul(
                            ctl_ps,
                            pb_sb[:, k * N + s * 128 : k * N + (s + 1) * 128],
                            pb_sb[:, 8 * N + k * NPC : 8 * N + (k + 1) * NPC],
                            start=(k == 0),
                            stop=(k == 7),
                        )
                    nc.vector.tensor_copy(ctl_sb[:, s, :], ctl_ps)
                agg_ps = p0b.tile([128, NPC], F32, tag="conv_ps")
                for s in range(2):
                    nc.tensor.matmul(
                        agg_ps,
                        x0th_sb[:, s, :],
                        ctl_sb[:, s, :],
                        start=(s == 0),
                        stop=False,
                    )
                # + conv of batch 0 for the local nodes -> agg_ps holds the
                # full pre-activation of batch 0 (minus conv_b)
                nc.tensor.matmul(
                    agg_ps,
                    pa(PA_RW4, OUT, F),
                    pa(PA_X0TL, NPC, F),
                    start=False,
                    stop=True,
                )
                # feat0[oc, n] = relu(conv0 + agg + conv_b): the correct
                # batch-0 feature column for every local node
                nc.scalar.activation(
                    feat0_sb,
                    agg_ps,
                    mybir.ActivationFunctionType.Relu,
                    bias=cb_ap,
                )

            bias_mm(0)
            for u, n0 in enumerate(range(0, NPC, U)):
                emit_unit(n0, 0, (u + 1) % 2)
            flush_acc()
            drain_half(0)

    nc.finalize()
    return nc


PW_HB, PW_V2W, PW_V2B, PW_V3W, PW_V3B = 0, 3, 67, 68, 132
PW_COLS = 133
PC_MGA, PC_MGB, PC_EM0, PC_EM1 = 0, 64, 128, 256
PC_COLS = 320
BT = B // M      # 128 batches per core
NQ = 2           # batch chunks pipelined through phase 2
BH = BT // NQ    # batch chunk per pipeline stage


def _build_phase2(repeat=1):
    nc = bacc.Bacc("TRN2")

    parts_d = nc.dram_tensor("parts", [NQ, 128, 3, M, BH], F32, kind="ExternalInput")
    pw_d = nc.dram_tensor("pw", [128, PW_COLS], F32, kind="ExternalInput")
    ot_d = nc.dram_tensor("ot", [NADV, BT], F32, kind="ExternalOutput")

    # structural constants, packed into one inline tensor
    dp = np.arange(NADV)
    mg = np.zeros((NADV, NDIV), np.float32)
    mg[dp, dp // PER] = -1.0 / PER           # negated group-mean matrix
    em = np.zeros((NDIV, NADV), np.float32)  # expand d -> (d,p)
    em[dp // PER, dp] = 1.0
    pc = np.zeros((128, PC_COLS), np.float32)
    pc[:, PC_MGA : PC_MGA + 64] = mg[:128]
    pc[0:64, PC_MGB : PC_MGB + 64] = mg[128:]
    pc[0:64, PC_EM0 : PC_EM0 + 128] = em[:, :128]
    pc[0:64, PC_EM1 : PC_EM1 + 64] = em[:, 128:]
    pc_d = nc.inline_tensor(np.ascontiguousarray(pc), name="pc")

    with tile.TileContext(nc) as tc:
      for rep in range(repeat):
        with ExitStack() as ctx:
            const = ctx.enter_context(tc.tile_pool(name=f"c2_{rep}", bufs=1))

            pp_sb = const.tile([128, NQ, 3, M, BH], F32, name="pp_sb")
            # parts first on the sync queue (critical path), one DMA per
            # batch chunk so the first chunk's pipeline starts early
            for q in range(NQ):
                nc.sync.dma_start(out=pp_sb[:, q], in_=parts_d[q])
            pw_sb = const.tile([128, PW_COLS], F32)
            nc.scalar.dma_start(out=pw_sb, in_=pw_d[:])
            pc_sb = const.tile([128, PC_COLS], F32)
            nc.scalar.dma_start(out=pc_sb, in_=pc_d[:])

            def pw(col, ncols, nrows=128):
                return pw_sb[0:nrows, col : col + ncols]

            def pcp(col, ncols, nrows=128):
                return pc_sb[0:nrows, col : col + ncols]

            work = ctx.enter_context(tc.tile_pool(name=f"work{rep}", bufs=2))
            psum = ctx.enter_context(
                tc.tile_pool(name=f"psum{rep}", bufs=2, space="PSUM")
            )
            ot0_sb = const.tile([128, BT], F32, name="ot0_sb")
            ot1_sb = const.tile([64, BT], F32, name="ot1_sb")

            for hb in range(NQ):
                pp = pp_sb[:, hb]
                bsl = slice(hb * BH, (hb + 1) * BH)
                # sum the 8 partials (tree; biases already folded in by
                # phase 1) then a single fused relu
                s1 = work.tile([128, 3, 4, BH], F32, name="s1")
                nc.vector.tensor_add(s1, pp[:, :, 0:4, :], pp[:, :, 4:8, :])
                s2 = work.tile([128, 3, 2, BH], F32, name="s2")
                nc.vector.tensor_add(s2, s1[:, :, 0:2, :], s1[:, :, 2:4, :])
                s3 = work.tile([128, 3, BH], F32, name="s3")
                nc.vector.tensor_add(s3, s2[:, :, 0, :], s2[:, :, 1, :])
                r_all = work.tile([128, 3, BH], F32, name="r_all")
                nc.vector.tensor_scalar_max(r_all, s3, 0.0)
                ar0 = r_all[:, 0, :]
                ar1 = r_all[:, 1, :]
                # ones row for the folded v2 bias
                nc.vector.memset(r_all[64:65, 2, :], 1.0)

                # v2 = relu(v1 @ v2w + v2b)   (v2b folded as K row 64)
                v2_ps = psum.tile([64, BH], F32, name="v2_ps")
                nc.tensor.matmul(v2_ps, pw(PW_V2W, 64, 65), r_all[0:65, 2, :])
                v2r = work.tile([65, BH], F32, name="v2r")
                nc.vector.tensor_scalar_max(v2r[0:64, :], v2_ps, 0.0)
                nc.vector.memset(v2r[64:65, :], 1.0)

                # val - mean(adv):  v3w^T@v2 + v3b + (-1/3 group sums)
                vm_ps = psum.tile([64, BH], F32, name="vm_ps")
                nc.tensor.matmul(
                    vm_ps, pw(PW_V3W, 64, 65), v2r, start=True, stop=False
                )
                nc.tensor.matmul(
                    vm_ps, pcp(PC_MGA, 64), ar0, start=False, stop=False
                )
                nc.tensor.matmul(
                    vm_ps, pcp(PC_MGB, 64, 64), ar1[0:64, :], start=False, stop=True
                )
                d1 = work.tile([64, BH], F32, name="d1")
                nc.vector.tensor_copy(d1, vm_ps)

                # expand to (d,p) rows and add adv
                dx0_ps = psum.tile([128, BH], F32, name="dx0_ps")
                nc.tensor.matmul(dx0_ps, pcp(PC_EM0, 128, 64), d1)
                dx1_ps = psum.tile([64, BH], F32, name="dx1_ps")
                nc.tensor.matmul(dx1_ps, pcp(PC_EM1, 64, 64), d1)
                nc.vector.tensor_add(ot0_sb[:, bsl], ar0, dx0_ps)
                nc.vector.tensor_add(ot1_sb[:, bsl], ar1[0:64, :], dx1_ps)
                nc.sync.dma_start(out=ot_d[0:128, bsl], in_=ot0_sb[:, bsl])
                nc.scalar.dma_start(
                    out=ot_d[128:NADV, bsl], in_=ot1_sb[:, bsl]
                )

    nc.finalize()
    return nc


def _get_programs(mode, repeat=1):
    key = (mode, repeat, CONV_U)
    if key not in _build_cache:
        _build_cache[key] = (_build_phase1(mode, repeat), _build_phase2(repeat))
    return _build_cache[key]


def _prep_phase1_inputs(inputs, mode):
    sd = _np_store_dtype(mode)
    x = np.ascontiguousarray(np.asarray(inputs["x"], np.float32))
    ei = np.asarray(inputs["edge_index"]).astype(np.int64)
    w1 = np.asarray(inputs["w1"], np.float32)
    b1 = np.asarray(inputs["b1"], np.float32)
    w2 = np.asarray(inputs["w2"], np.float32)
    b2 = np.asarray(inputs["b2"], np.float32)
    root_w = np.asarray(inputs["root_w"], np.float32)
    conv_b = np.asarray(inputs["conv_b"], np.float32)
    adv_w = np.asarray(inputs["adv_w"], np.float32)
    v1w = np.asarray(inputs["v1w"], np.float32)

    src_i, tgt_i = ei[0], ei[1]
    wfull = np.concatenate([adv_w, v1w], axis=1)  # [32768, 256]

    pa = np.zeros((128, PA_COLS), np.float32)
    pa[0:64, PA_W1T] = w1.reshape(64)
    pa[0:64, PA_B1T] = b1
    pa[:, PA_CB] = conv_b
    pa[:, PA_B2T : PA_B2T + F] = b2.reshape(F, OUT).T
    pa[0:F, PA_X0T : PA_X0T + N] = x[0].T
    pa[0:64, PA_W2 : PA_W2 + F * OUT] = w2
    for j in range(4):
        pa[32 * j : 32 * j + F, PA_RW4 : PA_RW4 + OUT] = root_w

    soh = (src_i[:, None] == np.arange(N)[None, :]).astype(np.float32)
    soh = soh.reshape(8, 128, N).transpose(1, 0, 2).reshape(128, 8 * N)

    adv_b = np.asarray(inputs["adv_b"], np.float32)
    v1b = np.asarray(inputs["v1b"], np.float32)
    brow = np.concatenate([adv_b, v1b])          # [256]

    in_maps = []
    for c in range(M):
        pac = pa.copy()
        pac[0:F, PA_X0TL : PA_X0TL + NPC] = x[0, NPC * c : NPC * (c + 1), :].T
        pa2 = np.zeros((128, P2_COLS), np.float32)
        for j in range(4):
            pa2[32 * j : 32 * j + F, P2_RW4 : P2_RW4 + OUT] = root_w
        if c == 0:
            pa2[0, P2_BROW : P2_BROW + AV] = brow
        pa2[0, P2_ONES : P2_ONES + 512] = 1.0
        pb = np.zeros((128, PB_COLS), np.float32)
        pb[:, 0 : 8 * N] = soh
        tohl = (tgt_i[:, None] == (NPC * c + np.arange(NPC))[None, :]).astype(
            np.float32
        )
        pb[:, 8 * N :] = tohl.reshape(8, 128, NPC).transpose(1, 0, 2).reshape(
            128, 8 * NPC
        )

        xw = np.zeros((128, 8, 2048), np.float32)
        xc = x[:, NPC * c : NPC * (c + 1), :]           # [B, 32, 8]
        xr = xc.transpose(1, 2, 0)                      # [32, 8, B]
        for j in range(4):
            # node 4g+j sits at partitions 32j..32j+F of group g
            xw[32 * j : 32 * j + F, :, 0:1024] = xr[j::4].transpose(1, 0, 2)
        rows = wfull[4096 * c : 4096 * (c + 1)]         # [4096, 256]
        xw[:, :, 1024:2048] = (
            rows.reshape(8, 4, 128, AV)
            .transpose(2, 0, 1, 3)
            .reshape(128, 8, 1024)
        )
        in_maps.append(
            {"pa": pac, "pa2": pa2.astype(sd), "pb": pb, "xw": xw.astype(sd)}
        )
    return in_maps


def _prep_phase2_inputs(inputs, pts):
    adv_b = np.asarray(inputs["adv_b"], np.float32)
    v1b = np.asarray(inputs["v1b"], np.float32)
    v2w = np.asarray(inputs["v2w"], np.float32)
    v2b = np.asarray(inputs["v2b"], np.float32)
    v3w = np.asarray(inputs["v3w"], np.float32)
    v3b = np.asarray(inputs["v3b"], np.float32)

    pw = np.zeros((128, PW_COLS), np.float32)
    pw[0:64, PW_V2W : PW_V2W + 64] = v2w
    pw[64, PW_V2W : PW_V2W + 64] = v2b
    pw[0:64, PW_V3W : PW_V3W + 64] = v3w
    pw[64, PW_V3W : PW_V3W + 64] = v3b

    in_maps = []
    for c in range(M):
        bsl = slice(BT * c, BT * (c + 1))
        stk = np.stack([p[:, bsl] for p in pts])              # [i, 256, BT]
        parts = np.zeros((NQ, 128, 3, M, BH), np.float32)
        for hb in range(NQ):
            hsl = slice(hb * BH, (hb + 1) * BH)
            parts[hb, :, 0] = stk[:, 0:128, hsl].transpose(1, 0, 2)
            parts[hb, 0:64, 1] = stk[:, 128:NADV, hsl].transpose(1, 0, 2)
            parts[hb, 0:64, 2] = stk[:, NADV:AV, hsl].transpose(1, 0, 2)
        in_maps.append({"parts": parts, "pw": pw})
    return in_maps


class _Runner:
    """Cached PJRT executor for one Bass program across the 8 cores.

    Mirrors bass2jax.run_bass_via_pjrt but keeps the jitted callable so
    repeat calls don't re-trace/re-lower, enabling benchmarking.
    """

    def __init__(self, nc):
        import jax
        from jax.sharding import Mesh, PartitionSpec, NamedSharding
        from jax.experimental.shard_map import shard_map
        from concourse import bass2jax

        bass2jax.install_neuronx_cc_hook()
        self.jax = jax
        self.nc = nc
        partition_name = (
            nc.partition_id_tensor.name if nc.partition_id_tensor else None
        )
        in_names, out_names, out_avals, zero_shapes = [], [], [], []
        for alloc in nc.m.functions[0].allocations:
            if not isinstance(alloc, mybir.MemoryLocationSet):
                continue
            name = alloc.memorylocations[0].name
            if alloc.kind == "ExternalInput":
                if name != partition_name:
                    in_names.append(name)
            elif alloc.kind == "ExternalOutput":
                shape = tuple(alloc.tensor_shape)
                dtype = mybir.dt.np(alloc.dtype)
                out_names.append(name)
                out_avals.append(jax.core.ShapedArray(shape, dtype))
                zero_shapes.append((shape, dtype))
        self.in_names, self.out_names = in_names, out_names
        self.out_avals, self.zero_shapes = out_avals, zero_shapes
        n_params, n_outs = len(in_names), len(out_names)
        self.n_params = n_params

        bind_names = in_names + out_names
        if partition_name is not None:
            bind_names = bind_names + [partition_name]

        def _body(*args):
            operands = list(args)
            if partition_name is not None:
                operands.append(bass2jax.partition_id_tensor())
            outs = bass2jax._bass_exec_p.bind(
                *operands,
                out_avals=tuple(out_avals),
                in_names=tuple(bind_names),
                out_names=tuple(out_names),
                lowering_input_output_aliases=(),
                sim_require_finite=True,
                sim_require_nnan=True,
                nc=nc,
            )
            return tuple(outs)

        devices = jax.devices()[:M]
        self.mesh = Mesh(np.asarray(devices), ("core",))
        spec = PartitionSpec("core")
        self.sharding = NamedSharding(self.mesh, spec)
        donate = tuple(range(n_params, n_params + n_outs))
        self.fn = jax.jit(
            shard_map(
                _body,
                mesh=self.mesh,
                in_specs=(spec,) * (n_params + n_outs),
                out_specs=(spec,) * n_outs,
                check_rep=False,
            ),
            donate_argnums=donate,
            keep_unused=True,
        )

    def _concat_inputs(self, in_maps):
        return [
            np.concatenate([np.asarray(m[name]) for m in in_maps], axis=0)
            for name in self.in_names
        ]

    def _zeros(self):
        return [np.zeros((M * s[0], *s[1:]), d) for s, d in self.zero_shapes]

    def _split(self, out_arrs):
        res = []
        for c in range(M):
            res.append(
                {
                    name: np.asarray(out_arrs[i]).reshape(M, *self.out_avals[i].shape)[c]
                    for i, name in enumerate(self.out_names)
                }
            )
        return res

    def run(self, in_maps):
        out_arrs = self.fn(*self._concat_inputs(in_maps), *self._zeros())
        return self._split(out_arrs)

    def bench(self, in_maps, iters=20):
        import time

        jax = self.jax
        dev_in = [
            jax.device_put(a, self.sharding) for a in self._concat_inputs(in_maps)
        ]
        times = []
        out_arrs = None
        for _ in range(iters):
            zeros = [jax.device_put(z, self.sharding) for z in self._zeros()]
            jax.block_until_ready(zeros)
            t0 = time.perf_counter()
            out_arrs = self.fn(*dev_in, *zeros)
            jax.block_until_ready(out_arrs)
            times.append(time.perf_counter() - t0)
        return self._split(out_arrs), times


_runner_cache = {}


def _get_runner(nc, key):
    if key not in _runner_cache:
        _runner_cache[key] = _Runner(nc)
    return _runner_cache[key]


def _run_sim(nc, in_maps):
    from concourse.bass_interp import CoreSim

    outs = []
    for im in in_maps:
        sim = CoreSim(nc)
        for k, v in im.items():
            sim.tensor(k)[:] = v
        sim.simulate()
        out_name = "pt" if "xw" in im else "ot"
        outs.append({out_name: np.array(sim.tensor(out_name))})
    return outs


def _run(inputs, mode=None, trace=False, backend="hw", bench_iters=0):
    mode = mode or MODE
    nc1, nc2 = _get_programs(mode)
    info = {}

    in_maps1 = _prep_phase1_inputs(inputs, mode)
    if backend == "sim":
        res1 = _run_sim(nc1, in_maps1)
    else:
        runner1 = _get_runner(nc1, ("p1", mode, CONV_U))
        if bench_iters:
            res1, times = runner1.bench(in_maps1, bench_iters)
            info["phase1_ns"] = int(min(times) * 1e9)
            info["phase1_mean_ns"] = float(np.mean(times) * 1e9)
        else:
            res1 = runner1.run(in_maps1)
    pts = [np.asarray(res1[c]["pt"], np.float32) for c in range(M)]

    in_maps2 = _prep_phase2_inputs(inputs, pts)
    if backend == "sim":
        res2 = _run_sim(nc2, in_maps2)
    else:
        runner2 = _get_runner(nc2, ("p2",))
        if bench_iters:
            res2, times = runner2.bench(in_maps2, bench_iters)
            info["phase2_ns"] = int(min(times) * 1e9)
            info["phase2_mean_ns"] = float(np.mean(times) * 1e9)
        else:
            res2 = runner2.run(in_maps2)

    BT = B // M
    out = np.empty((B, NDIV, PER), np.float32)
    for c in range(M):
        ot = np.asarray(res2[c]["ot"], np.float32)  # [192, 128]
        out[BT * c : BT * (c + 1)] = ot.T.reshape(BT, NDIV, PER)
    return out, info


def _p25(ts):
    ts = sorted(ts)
    return ts[max(0, len(ts) // 4)]


def bench_hw(inputs, mode=None, big_rep=9, iters=12):
    """Differential HW timing: (T(R) - T(1)) / (R - 1) cancels the axon
    launch overhead and measures the true per-pass device time.  Uses the
    25th percentile (the min is occasionally glitchy on the relay)."""
    mode = mode or MODE
    in_maps1 = _prep_phase1_inputs(inputs, mode)
    res = {}
    est = {}
    for r in (1, big_rep):
        nc1, _ = _get_programs(mode, r)
        runner = _get_runner(nc1, ("p1", mode, CONV_U, r))
        out1, times = runner.bench(in_maps1, iters)
        est[r] = _p25(times)
    res["phase1_ns"] = (est[big_rep] - est[1]) / (big_rep - 1) * 1e9
    res["phase1_launch_ns"] = est[1] * 1e9

    pts = [np.asarray(o["pt"], np.float32) for o in out1]
    in_maps2 = _prep_phase2_inputs(inputs, pts)
    for r in (1, big_rep):
        _, nc2 = _get_programs(mode, r)
        runner = _get_runner(nc2, ("p2", r))
        _, times = runner.bench(in_maps2, iters)
        est[r] = _p25(times)
    res["phase2_ns"] = (est[big_rep] - est[1]) / (big_rep - 1) * 1e9
    res["phase2_launch_ns"] = est[1] * 1e9
    return res


def kernel(**inputs):
    out, _ = _run(inputs)
    return out



# revision 11
# speedup vs baseline: 1.0382x; 1.0382x over previous
"""Trainium2 Bass kernel for nn_BHSDuelingDQN (gnn_message_passing).

Math notes (validated vs reference to fp32 precision):
  - The edge MLP input is ones(E,1), so every edge shares one theta [F,OUT]:
        theta = (relu(w1[0]+b1) @ w2 + b2).reshape(F, OUT)
  - edge_index values live in [0, N), so the gather/scatter-add only touches
    batch 0 of flat=[B*N,F].  With C[t,s] = #edges(src=s, tgt=t):
        agg(batch0) = C @ (x[0] @ theta)
    which turns the whole message passing into dense matmuls.

Sharding: phase 1 is node-sharded (each of 8 cores owns 32 of 256 nodes and
computes partial pre-activations of adv/v1 for all 1024 batches over its
4096 feature rows).  Phase 2 is batch-sharded (each core sums the 8 partials
for its 128 batches and runs the small value-head + dueling combine).  The
host only slices / concatenates / transposes arrays between phases.

Phase 1 conv trick: x is packed dense ([128 part] = 16 nodes x 8 feats) and
the per-node K=8 conv matmul uses a K=32 zero-padded stationary (root_w at
sub-offset 8j of a 32-row tile, zeros elsewhere) built on device, so no DMA
bytes are wasted on layout padding.

Layouts are transposed ([feature, batch]) throughout so no on-device
transposes are ever needed.
"""

import os
from contextlib import ExitStack

import numpy as np

import concourse.bacc as bacc
import concourse.bass as bass
import concourse.mybir as mybir
import concourse.tile as tile
from concourse import masks
from concourse.bass_utils import run_bass_kernel_spmd  # noqa: F401  (spmd entry)

F32 = mybir.dt.float32
BF16 = mybir.dt.bfloat16

B, N, F, E, OUT, NDIV, PER = 1024, 256, 8, 1024, 128, 64, 3
NADV = NDIV * PER            # 192
AV = NADV + 64               # 256 fused output cols of phase 1 (adv | v1)
M = 8                        # cores
NPC = N // M                 # 32 nodes per core

# hot-path matmul dtype mode: 'f32' (exact, 4x slower PE), 'f32r' (full rate,
# reduced-precision fp32 path), 'bf16' (half DMA traffic)
MODE = os.environ.get("BASS_KERNEL_MODE", "f32r")
NQ = int(os.environ.get("BASS_P2_NQ", "2"))   # phase-2 batch chunks
WARMUP = int(os.environ.get("BASS_WARMUP", "0"))  # dummy PE warmup matmuls

_build_cache = {}


def _np_bf16():
    import ml_dtypes

    return np.dtype(ml_dtypes.bfloat16)


def _np_store_dtype(mode):
    if mode == "bf16":
        return _np_bf16()
    return np.dtype(np.float32)


def _store_dt(mode):
    if mode == "bf16":
        return BF16
    if mode == "f32r":
        return mybir.dt.float32r  # fp32 bytes, PE full-rate reduced-precision
    return F32


# pa column layout: crit block first (needed before first conv), rest after.
PA_W1T, PA_B1T, PA_CB, PA_RW, PA_B2T = 0, 1, 2, 3, 131
PA_CRIT = 139
PA_X0T, PA_X0TL, PA_W2 = 139, 395, 427
PA_COLS = 427 + 64 * 16  # w2 is [64, F*OUT=1024]
PB_COLS = 8 * N + 8 * NPC  # soh | tohl


def _build_phase1(mode, repeat=1):
    store_dt = _store_dt(mode)
    nc = bacc.Bacc("TRN2")

    pa_d = nc.dram_tensor("pa", [128, PA_COLS], F32, kind="ExternalInput")
    stw_d = nc.dram_tensor("stw", [128, 4 * OUT], store_dt, kind="ExternalInput")
    xd_d = nc.dram_tensor("xd", [128, 2, B], store_dt, kind="ExternalInput")
    wf_d = nc.dram_tensor("wf", [128, NPC, AV], store_dt, kind="ExternalInput")
    pb_d = nc.dram_tensor("pb", [128, PB_COLS], BF16, kind="ExternalInput")
    pt_d = nc.dram_tensor("pt", [AV, B], BF16, kind="ExternalOutput")

    with tile.TileContext(nc) as tc:
      for rep in range(repeat):
        with ExitStack() as ctx:
            const = ctx.enter_context(tc.tile_pool(name=f"const{rep}", bufs=1))

            pa_sb = const.tile([128, PA_COLS], F32)
            st_sb = const.tile([128, 4 * OUT], store_dt)
            xd_sb = const.tile([128, 2, B], store_dt)
            wf_sb = const.tile([128, NPC, AV], store_dt)
            pb_sb = const.tile([128, PB_COLS], BF16)

            # scalar queue: small critical pack first, then the rest of pa
            nc.scalar.dma_start(out=pa_sb[:, 0:PA_CRIT], in_=pa_d[:, 0:PA_CRIT])
            nc.scalar.dma_start(out=st_sb, in_=stw_d[:])
            nc.scalar.dma_start(
                out=pa_sb[:, PA_CRIT:PA_COLS], in_=pa_d[:, PA_CRIT:PA_COLS]
            )
            # sync queue: streaming loads in consumption order.
            # h=1 sweep (batch 512:1024) runs first.
            nc.sync.dma_start(out=xd_sb[:, 0, 512:B], in_=xd_d[:, 0, 512:B])
            nc.sync.dma_start(out=wf_sb[:, 0:8], in_=wf_d[:, 0:8])
            nc.sync.dma_start(out=xd_sb[:, 1, 512:B], in_=xd_d[:, 1, 512:B])
            nc.sync.dma_start(out=wf_sb[:, 8:16], in_=wf_d[:, 8:16])
            nc.sync.dma_start(out=pb_sb, in_=pb_d[:])
            nc.sync.dma_start(out=xd_sb[:, 0, 0:512], in_=xd_d[:, 0, 0:512])
            nc.sync.dma_start(out=xd_sb[:, 1, 0:512], in_=xd_d[:, 1, 0:512])
            nc.sync.dma_start(out=wf_sb[:, 16:24], in_=wf_d[:, 16:24])
            nc.sync.dma_start(out=wf_sb[:, 24:NPC], in_=wf_d[:, 24:NPC])

            def pa(col, ncols, nrows=128):
                return pa_sb[0:nrows, col : col + ncols]

            cb_ap = pa(PA_CB, 1)

            ident_sb = const.tile([128, 128], F32)
            masks.make_identity(nc, ident_sb)

            h_sb = const.tile([64, 1], F32)
            thT_sb = const.tile([128, F], F32)
            th_sb = const.tile([F, OUT], F32)
            x0th_sb = const.tile([128, 2, OUT], F32)
            ctl_sb = const.tile([128, 2, NPC], F32)
            feat0_sb = const.tile([128, NPC], store_dt)

            # --- phase 0a: theta and x0@theta (exact fp32) --------------
            with tc.tile_pool(name=f"p0a{rep}", bufs=1, space="PSUM") as p0a:
                # h = relu(w1 + b1) on DVE (no activation-table dep)
                nc.vector.tensor_scalar(
                    h_sb,
                    pa(PA_W1T, 1, 64),
                    pa(PA_B1T, 1, 64),
                    0.0,
                    mybir.AluOpType.add,
                    mybir.AluOpType.max,
                )
                # thetaT[oc, f] = w2[:, f*128+oc]^T @ h   (8 N=1 matmuls)
                thT_ps = p0a.tile([128, F], F32, tag="p0")
                for f in range(F):
                    nc.tensor.matmul(
                        thT_ps[:, f : f + 1],
                        pa(PA_W2 + f * OUT, OUT, 64),
                        h_sb,
                    )
                nc.vector.tensor_add(thT_sb, thT_ps, pa(PA_B2T, F))
                # theta [f, oc] via PE transpose
                th_ps = p0a.tile([F, OUT], F32, tag="p0")
                nc.tensor.transpose(th_ps, thT_sb[:, 0:F], ident_sb)
                nc.vector.tensor_copy(th_sb, th_ps)
                # x0 @ theta  [256 src, 128 oc]
                for s in range(2):
                    x0th_ps = p0a.tile(
                        [128, OUT], F32, name=f"x0th_ps{s}", tag="p0"
                    )
                    nc.tensor.matmul(
                        x0th_ps, pa(PA_X0T + s * 128, 128, F), th_sb
                    )
                    nc.vector.tensor_copy(x0th_sb[:, s, :], x0th_ps)

            # --- main pools --------------------------------------------
            acc_pool = ctx.enter_context(
                tc.tile_pool(name=f"accp{rep}", bufs=1, space="PSUM")
            )
            conv_pool = ctx.enter_context(
                tc.tile_pool(name=f"convp{rep}", bufs=3, space="PSUM")
            )
            feat_pool = ctx.enter_context(tc.tile_pool(name=f"featp{rep}", bufs=4))
            out_pool = ctx.enter_context(tc.tile_pool(name=f"outp{rep}", bufs=1))
            acc_ps = [acc_pool.tile([128, B], F32, name=f"acc{m}") for m in range(2)]
            ob0 = out_pool.tile([128, B], BF16, name="ob0")
            ob1 = out_pool.tile([128, B], BF16, name="ob1")

            pending = None

            def emit_node(n, h, parity):
                # conv for node n, half h; zero-padded K=32 stationary
                nonlocal pending
                g2, r = n // 16, n % 16
                a, j = r // 4, r % 4
                conv_ps = conv_pool.tile([128, 512], F32, name="conv_ps")
                nc.tensor.matmul(
                    conv_ps,
                    st_sb[32 * a : 32 * a + 32, OUT * j : OUT * (j + 1)],
                    xd_sb[32 * a : 32 * a + 32, g2, h * 512 : (h + 1) * 512],
                    tile_position=(32 * a, 0),
                )
                feat_sb = feat_pool.tile([128, 512], store_dt, name="feat_sb")
                if parity == 0:
                    nc.scalar.activation(
                        feat_sb,
                        conv_ps,
                        mybir.ActivationFunctionType.Relu,
                        bias=cb_ap,
                    )
                else:
                    nc.vector.tensor_scalar(
                        feat_sb,
                        conv_ps,
                        cb_ap,
                        0.0,
                        mybir.AluOpType.add,
                        mybir.AluOpType.max,
                    )
                if h == 0:
                    # batch 0 lives in column 0: overwrite this node's
                    # first column with the precomputed feat0 value
                    nc.vector.tensor_copy(
                        feat_sb[:, 0:1], feat0_sb[:, n : n + 1]
                    )
                flush_acc()
                pending = (n, h, feat_sb)

            def flush_acc():
                nonlocal pending
                if pending is not None:
                    pn, ph, pf = pending
                    for m in range(2):
                        nc.tensor.matmul(
                            acc_ps[m][:, ph * 512 : (ph + 1) * 512],
                            wf_sb[:, pn, m * 128 : (m + 1) * 128],
                            pf,
                            start=(pn == 0),
                            stop=(pn == NPC - 1),
                        )
                    pending = None

            def drain_half(h):
                sl = slice(h * 512, (h + 1) * 512)
                nc.vector.tensor_copy(ob0[:, sl], acc_ps[0][:, sl])
                nc.scalar.activation(
                    ob1[:, sl],
                    acc_ps[1][:, sl],
                    mybir.ActivationFunctionType.Copy,
                )
                nc.sync.dma_start(out=pt_d[0:128, sl], in_=ob0[:, sl])
                nc.scalar.dma_start(out=pt_d[128:AV, sl], in_=ob1[:, sl])

            for n in range(NPC):
                emit_node(n, 1, n % 2)
            flush_acc()
            drain_half(1)

            # --- phase 0b: local count matrix and agg (needs pb) --------
            p0b = conv_pool
            for s in range(2):
                ctl_ps = p0b.tile(
                    [128, NPC], F32, name=f"ctl_ps{s}", tag="conv_ps"
                )
                for k in range(8):
                    nc.tensor.matmul(
                        ctl_ps,
                        pb_sb[:, k * N + s * 128 : k * N + (s + 1) * 128],
                        pb_sb[:, 8 * N + k * NPC : 8 * N + (k + 1) * NPC],
                        start=(k == 0),
                        stop=(k == 7),
                    )
                nc.vector.tensor_copy(ctl_sb[:, s, :], ctl_ps)
            agg_ps = p0b.tile([128, NPC], F32, tag="conv_ps")
            for s in range(2):
                nc.tensor.matmul(
                    agg_ps,
                    x0th_sb[:, s, :],
                    ctl_sb[:, s, :],
                    start=(s == 0),
                    stop=False,
                )
            # + conv of batch 0 for the local nodes -> agg_ps holds the
            # full pre-activation of batch 0 (minus conv_b)
            nc.tensor.matmul(
                agg_ps,
                pa(PA_RW, OUT, F),
                pa(PA_X0TL, NPC, F),
                start=False,
                stop=True,
            )
            # feat0[oc, n] = relu(conv0 + agg + conv_b): the correct
            # batch-0 feature column for every local node
            nc.scalar.activation(
                feat0_sb,
                agg_ps,
                mybir.ActivationFunctionType.Relu,
                bias=cb_ap,
            )

            for n in range(NPC):
                emit_node(n, 0, (n + 1) % 2)
            flush_acc()
            drain_half(0)

    nc.finalize()
    return nc


BT = B // M      # 128 batches per core
BH = BT // NQ    # batch chunk per pipeline stage

# pwb (bf16 weights) column layout
PWB_V2WZ, PWB_V3 = 0, 64
PWB_COLS = 128
# pcb (bf16 structural consts) column layout
PCB_ID, PCB_MGA, PCB_MGB, PCB_EM0, PCB_EM1 = 0, 128, 192, 256, 384
PCB_COLS = 448
# pwf (f32 per-partition biases) columns: bias fb0 | bias fb1 | v2b
PWF_COLS = 3


def _build_phase2(repeat=1):
    nc = bacc.Bacc("TRN2")

    parts_d = nc.dram_tensor(
        "parts", [128, NQ, M, 2, BH], BF16, kind="ExternalInput"
    )
    pwb_d = nc.dram_tensor("pwb", [128, PWB_COLS], BF16, kind="ExternalInput")
    pcb_d = nc.dram_tensor("pcb", [128, PCB_COLS], BF16, kind="ExternalInput")
    pwf_d = nc.dram_tensor("pwf", [128, PWF_COLS], F32, kind="ExternalInput")
    ot_d = nc.dram_tensor("ot", [NADV, BT], F32, kind="ExternalOutput")

    with tile.TileContext(nc) as tc:
      for rep in range(repeat):
        with ExitStack() as ctx:
            const = ctx.enter_context(tc.tile_pool(name=f"c2_{rep}", bufs=1))

            pp_sb = const.tile([128, NQ, M, 2, BH], BF16, name="pp_sb")
            for q in range(NQ):
                nc.sync.dma_start(out=pp_sb[:, q], in_=parts_d[:, q])
            pwb_sb = const.tile([128, PWB_COLS], BF16)
            nc.scalar.dma_start(out=pwb_sb, in_=pwb_d[:])
            pcb_sb = const.tile([128, PCB_COLS], BF16)
            nc.scalar.dma_start(out=pcb_sb, in_=pcb_d[:])
            pwf_sb = const.tile([128, PWF_COLS], F32)
            nc.scalar.dma_start(out=pwf_sb, in_=pwf_d[:])

            work = ctx.enter_context(tc.tile_pool(name=f"work{rep}", bufs=2))
            psum = ctx.enter_context(
                tc.tile_pool(name=f"psum{rep}", bufs=2, space="PSUM")
            )
            ot0_sb = const.tile([128, BT], F32, name="ot0_sb")
            ot1_sb = const.tile([64, BT], F32, name="ot1_sb")

            for hb in range(NQ):
                bsl = slice(hb * BH, (hb + 1) * BH)
                # sum the 8 partials on the PE (identity-stationary
                # accumulation into PSUM), both feature blocks at once
                s_ps = psum.tile([128, 2, BH], F32, name="s_ps")
                for c in range(M):
                    nc.tensor.matmul(
                        s_ps,
                        pcb_sb[:, PCB_ID : PCB_ID + 128],
                        pp_sb[:, hb, c],
                        start=(c == 0),
                        stop=(c == M - 1),
                    )
                # ar = relu(s + head_bias), per feature block
                ar = work.tile([128, 2, BH], BF16, name="ar")
                for fb in range(2):
                    nc.vector.tensor_scalar(
                        ar[:, fb],
                        s_ps[:, fb],
                        pwf_sb[:, fb : fb + 1],
                        0.0,
                        mybir.AluOpType.add,
                        mybir.AluOpType.max,
                    )
                # small PSUM scratch: v2 | vm | dx0 | dx1 packed in one bank
                sm_ps = psum.tile([128, 4, BH], F32, name="sm_ps")
                v2_ps = sm_ps[0:64, 0]
                vm_ps = sm_ps[0:64, 1]
                dx0_ps = sm_ps[:, 2]
                dx1_ps = sm_ps[0:64, 3]
                # v2 = relu(v2w^T v1 + v2b); v1 sits at rows 64:128 of fb1
                nc.tensor.matmul(
                    v2_ps, pwb_sb[:, PWB_V2WZ : PWB_V2WZ + 64], ar[:, 1]
                )
                v2r = work.tile([65, BH], BF16, name="v2r")
                nc.vector.tensor_scalar(
                    v2r[0:64],
                    v2_ps,
                    pwf_sb[0:64, 2:3],
                    0.0,
                    mybir.AluOpType.add,
                    mybir.AluOpType.max,
                )
                nc.vector.memset(v2r[64:65], 1.0)

                # vm = v3w^T v2 + v3b - (1/3) group-sums(adv)
                nc.tensor.matmul(
                    vm_ps,
                    pwb_sb[0:65, PWB_V3 : PWB_V3 + 64],
                    v2r,
                    start=True,
                    stop=False,
                )
                nc.tensor.matmul(
                    vm_ps,
                    pcb_sb[:, PCB_MGA : PCB_MGA + 64],
                    ar[:, 0],
                    start=False,
                    stop=False,
                )
                nc.tensor.matmul(
                    vm_ps,
                    pcb_sb[:, PCB_MGB : PCB_MGB + 64],
                    ar[:, 1],
                    start=False,
                    stop=True,
                )
                d1 = work.tile([64, BH], BF16, name="d1")
                nc.vector.tensor_copy(d1, vm_ps)

                # expand to (d,p) rows and add adv
                nc.tensor.matmul(
                    dx0_ps, pcb_sb[0:64, PCB_EM0 : PCB_EM0 + 128], d1
                )
                nc.tensor.matmul(
                    dx1_ps, pcb_sb[0:64, PCB_EM1 : PCB_EM1 + 64], d1
                )
                nc.vector.tensor_add(ot0_sb[:, bsl], ar[:, 0], dx0_ps)
                nc.vector.tensor_add(ot1_sb[:, bsl], ar[0:64, 1], dx1_ps)
                nc.sync.dma_start(out=ot_d[0:128, bsl], in_=ot0_sb[:, bsl])
                nc.scalar.dma_start(out=ot_d[128:NADV, bsl], in_=ot1_sb[:, bsl])

    nc.finalize()
    return nc


def _get_programs(mode, repeat=1):
    key = (mode, repeat, NQ)
    if key not in _build_cache:
        _build_cache[key] = (_build_phase1(mode, repeat), _build_phase2(repeat))
    return _build_cache[key]


def _prep_phase1_inputs(inputs, mode):
    sd = _np_store_dtype(mode)
    bf = _np_bf16()
    x = np.ascontiguousarray(np.asarray(inputs["x"], np.float32))
    ei = np.asarray(inputs["edge_index"]).astype(np.int64)
    w1 = np.asarray(inputs["w1"], np.float32)
    b1 = np.asarray(inputs["b1"], np.float32)
    w2 = np.asarray(inputs["w2"], np.float32)
    b2 = np.asarray(inputs["b2"], np.float32)
    root_w = np.asarray(inputs["root_w"], np.float32)
    conv_b = np.asarray(inputs["conv_b"], np.float32)
    adv_w = np.asarray(inputs["adv_w"], np.float32)
    v1w = np.asarray(inputs["v1w"], np.float32)

    src_i, tgt_i = ei[0], ei[1]
    wfull = np.concatenate([adv_w, v1w], axis=1)  # [32768, 256]

    pa = np.zeros((128, PA_COLS), np.float32)
    pa[0:64, PA_W1T] = w1.reshape(64)
    pa[0:64, PA_B1T] = b1
    pa[:, PA_CB] = conv_b
    pa[0:F, PA_RW : PA_RW + OUT] = root_w
    pa[:, PA_B2T : PA_B2T + F] = b2.reshape(F, OUT).T
    pa[0:F, PA_X0T : PA_X0T + N] = x[0].T
    pa[0:64, PA_W2 : PA_W2 + F * OUT] = w2

    soh = (src_i[:, None] == np.arange(N)[None, :]).astype(np.float32)
    soh = soh.reshape(8, 128, N).transpose(1, 0, 2).reshape(128, 8 * N)

    # zero-padded conv stationary: stw[32a+8j+f, 128j+oc] = root_w[f, oc]
    stw = np.zeros((128, 4 * OUT), np.float32)
    for a in range(4):
        for j in range(4):
            stw[32 * a + 8 * j : 32 * a + 8 * j + F, OUT * j : OUT * (j + 1)] = (
                root_w
            )
    stw = stw.astype(sd)

    in_maps = []
    for c in range(M):
        pac = pa.copy()
        pac[0:F, PA_X0TL : PA_X0TL + NPC] = x[0, NPC * c : NPC * (c + 1), :].T
        pb = np.zeros((128, PB_COLS), np.float32)
        pb[:, 0 : 8 * N] = soh
        tohl = (tgt_i[:, None] == (NPC * c + np.arange(NPC))[None, :]).astype(
            np.float32
        )
        pb[:, 8 * N :] = tohl.reshape(8, 128, NPC).transpose(1, 0, 2).reshape(
            128, 8 * NPC
        )

        # dense x pack: node n -> rows 32a+8j (a=(n%16)//4, j=n%4), slab n//16
        xc = x[:, NPC * c : NPC * (c + 1), :]           # [B, 32, 8]
        xr = xc.transpose(1, 2, 0)                      # [32, 8, B]
        xd = (
            xr.reshape(2, 4, 4, F, B)
            .transpose(1, 2, 3, 0, 4)
            .reshape(128, 2, B)
        )
        wfc = (
            wfull[4096 * c : 4096 * (c + 1)]            # [4096, 256]
            .reshape(NPC, 128, AV)
            .transpose(1, 0, 2)                         # [128, 32, 256]
        )
        in_maps.append(
            {
                "pa": pac,
                "stw": stw,
                "xd": xd.astype(sd),
                "wf": np.ascontiguousarray(wfc).astype(sd),
                "pb": pb.astype(bf),
            }
        )
    return in_maps


def _prep_phase2_inputs(inputs, pts):
    bf = _np_bf16()
    adv_b = np.asarray(inputs["adv_b"], np.float32)
    v1b = np.asarray(inputs["v1b"], np.float32)
    v2w = np.asarray(inputs["v2w"], np.float32)
    v2b = np.asarray(inputs["v2b"], np.float32)
    v3w = np.asarray(inputs["v3w"], np.float32)
    v3b = np.asarray(inputs["v3b"], np.float32)

    pwb = np.zeros((128, PWB_COLS), np.float32)
    pwb[64:128, PWB_V2WZ : PWB_V2WZ + 64] = v2w
    pwb[0:64, PWB_V3 : PWB_V3 + 64] = v3w
    pwb[64, PWB_V3 : PWB_V3 + 64] = v3b

    pwf = np.zeros((128, PWF_COLS), np.float32)
    pwf[:, 0] = adv_b[0:128]
    pwf[0:64, 1] = adv_b[128:NADV]
    pwf[64:128, 1] = v1b
    pwf[0:64, 2] = v2b

    dp = np.arange(NADV)
    mg = np.zeros((NADV, NDIV), np.float32)
    mg[dp, dp // PER] = -1.0 / PER           # negated group-mean matrix
    em = np.zeros((NDIV, NADV), np.float32)  # expand d -> (d,p)
    em[dp // PER, dp] = 1.0
    pcb = np.zeros((128, PCB_COLS), np.float32)
    pcb[:, PCB_ID : PCB_ID + 128] = np.eye(128)
    pcb[:, PCB_MGA : PCB_MGA + 64] = mg[0:128]
    pcb[0:64, PCB_MGB : PCB_MGB + 64] = mg[128:NADV]
    pcb[0:64, PCB_EM0 : PCB_EM0 + 128] = em[:, 0:128]
    pcb[0:64, PCB_EM1 : PCB_EM1 + 64] = em[:, 128:NADV]

    pts_f = [np.asarray(p, np.float32) for p in pts]    # [256, 1024] each
    in_maps = []
    for c in range(M):
        parts = np.zeros((128, NQ, M, 2, BH), np.float32)
        for cc in range(M):
            blk = pts_f[cc][:, BT * c : BT * (c + 1)]   # [256, 128]
            parts[:, :, cc, 0, :] = blk[0:128].reshape(128, NQ, BH)
            parts[:, :, cc, 1, :] = blk[128:256].reshape(128, NQ, BH)
        in_maps.append(
            {
                "parts": parts.astype(bf),
                "pwb": pwb.astype(bf),
                "pcb": pcb.astype(bf),
                "pwf": pwf,
            }
        )
    return in_maps


class _Runner:
    """Cached PJRT executor for one Bass program across the 8 cores.

    Mirrors bass2jax.run_bass_via_pjrt but keeps the jitted callable so
    repeat calls don't re-trace/re-lower, enabling benchmarking.
    """

    def __init__(self, nc):
        import jax
        from jax.sharding import Mesh, PartitionSpec, NamedSharding
        from jax.experimental.shard_map import shard_map
        from concourse import bass2jax

        bass2jax.install_neuronx_cc_hook()
        self.jax = jax
        self.nc = nc
        partition_name = (
            nc.partition_id_tensor.name if nc.partition_id_tensor else None
        )
        in_names, out_names, out_avals, zero_shapes = [], [], [], []
        for alloc in nc.m.functions[0].allocations:
            if not isinstance(alloc, mybir.MemoryLocationSet):
                continue
            name = alloc.memorylocations[0].name
            if alloc.kind == "ExternalInput":
                if name != partition_name:
                    in_names.append(name)
            elif alloc.kind == "ExternalOutput":
                shape = tuple(alloc.tensor_shape)
                dtype = mybir.dt.np(alloc.dtype)
                out_names.append(name)
                out_avals.append(jax.core.ShapedArray(shape, dtype))
                zero_shapes.append((shape, dtype))
        self.in_names, self.out_names = in_names, out_names
        self.out_avals, self.zero_shapes = out_avals, zero_shapes
        n_params, n_outs = len(in_names), len(out_names)
        self.n_params = n_params

        bind_names = in_names + out_names
        if partition_name is not None:
            bind_names = bind_names + [partition_name]

        def _body(*args):
            operands = list(args)
            if partition_name is not None:
                operands.append(bass2jax.partition_id_tensor())
            outs = bass2jax._bass_exec_p.bind(
                *operands,
                out_avals=tuple(out_avals),
                in_names=tuple(bind_names),
                out_names=tuple(out_names),
                lowering_input_output_aliases=(),
                sim_require_finite=True,
                sim_require_nnan=True,
                nc=nc,
            )
            return tuple(outs)

        devices = jax.devices()[:M]
        self.mesh = Mesh(np.asarray(devices), ("core",))
        spec = PartitionSpec("core")
        self.sharding = NamedSharding(self.mesh, spec)
        donate = tuple(range(n_params, n_params + n_outs))
        self.fn = jax.jit(
            shard_map(
                _body,
                mesh=self.mesh,
                in_specs=(spec,) * (n_params + n_outs),
                out_specs=(spec,) * n_outs,
                check_rep=False,
            ),
            donate_argnums=donate,
            keep_unused=True,
        )

    def _concat_inputs(self, in_maps):
        return [
            np.concatenate([np.asarray(m[name]) for m in in_maps], axis=0)
            for name in self.in_names
        ]

    def _zeros(self):
        return [np.zeros((M * s[0], *s[1:]), d) for s, d in self.zero_shapes]

    def _split(self, out_arrs):
        res = []
        for c in range(M):
            res.append(
                {
                    name: np.asarray(out_arrs[i]).reshape(M, *self.out_avals[i].shape)[c]
                    for i, name in enumerate(self.out_names)
                }
            )
        return res

    def run(self, in_maps):
        out_arrs = self.fn(*self._concat_inputs(in_maps), *self._zeros())
        return self._split(out_arrs)

    def bench(self, in_maps, iters=20):
        import time

        jax = self.jax
        dev_in = [
            jax.device_put(a, self.sharding) for a in self._concat_inputs(in_maps)
        ]
        times = []
        out_arrs = None
        for _ in range(iters):
            zeros = [jax.device_put(z, self.sharding) for z in self._zeros()]
            jax.block_until_ready(zeros)
            t0 = time.perf_counter()
            out_arrs = self.fn(*dev_in, *zeros)
            jax.block_until_ready(out_arrs)
            times.append(time.perf_counter() - t0)
        return self._split(out_arrs), times


_runner_cache = {}


def _get_runner(nc, key):
    if key not in _runner_cache:
        _runner_cache[key] = _Runner(nc)
    return _runner_cache[key]


def _run_sim(nc, in_maps):
    from concourse.bass_interp import CoreSim

    outs = []
    for im in in_maps:
        sim = CoreSim(nc)
        for k, v in im.items():
            sim.tensor(k)[:] = v
        sim.simulate()
        out_name = "pt" if "xd" in im else "ot"
        outs.append({out_name: np.array(sim.tensor(out_name))})
    return outs


def _run(inputs, mode=None, trace=False, backend="hw", bench_iters=0):
    mode = mode or MODE
    nc1, nc2 = _get_programs(mode)
    info = {}

    in_maps1 = _prep_phase1_inputs(inputs, mode)
    if backend == "sim":
        res1 = _run_sim(nc1, in_maps1)
    else:
        runner1 = _get_runner(nc1, ("p1", mode))
        if bench_iters:
            res1, times = runner1.bench(in_maps1, bench_iters)
            info["phase1_ns"] = int(min(times) * 1e9)
            info["phase1_mean_ns"] = float(np.mean(times) * 1e9)
        else:
            res1 = runner1.run(in_maps1)
    pts = [res1[c]["pt"] for c in range(M)]

    in_maps2 = _prep_phase2_inputs(inputs, pts)
    if backend == "sim":
        res2 = _run_sim(nc2, in_maps2)
    else:
        runner2 = _get_runner(nc2, ("p2", NQ))
        if bench_iters:
            res2, times = runner2.bench(in_maps2, bench_iters)
            info["phase2_ns"] = int(min(times) * 1e9)
            info["phase2_mean_ns"] = float(np.mean(times) * 1e9)
        else:
            res2 = runner2.run(in_maps2)

    out = np.empty((B, NDIV, PER), np.float32)
    for c in range(M):
        ot = np.asarray(res2[c]["ot"], np.float32)  # [192, 128]
        out[BT * c : BT * (c + 1)] = ot.T.reshape(BT, NDIV, PER)
    return out, info


def _p25(ts):
    ts = sorted(ts)
    return ts[max(0, len(ts) // 4)]


def bench_hw(inputs, mode=None, big_rep=9, iters=12):
    """Differential HW timing: (T(R) - T(1)) / (R - 1) cancels the axon
    launch overhead and measures the true per-pass device time.  Uses the
    25th percentile (the min is occasionally glitchy on the relay)."""
    mode = mode or MODE
    in_maps1 = _prep_phase1_inputs(inputs, mode)
    res = {}
    est = {}
    for r in (1, big_rep):
        nc1, _ = _get_programs(mode, r)
        runner = _get_runner(nc1, ("p1", mode, r))
        out1, times = runner.bench(in_maps1, iters)
        est[r] = _p25(times)
    res["phase1_ns"] = (est[big_rep] - est[1]) / (big_rep - 1) * 1e9
    res["phase1_launch_ns"] = est[1] * 1e9

    pts = [o["pt"] for o in out1]
    in_maps2 = _prep_phase2_inputs(inputs, pts)
    for r in (1, big_rep):
        _, nc2 = _get_programs(mode, r)
        runner = _get_runner(nc2, ("p2", NQ, r))
        _, times = runner.bench(in_maps2, iters)
        est[r] = _p25(times)
    res["phase2_ns"] = (est[big_rep] - est[1]) / (big_rep - 1) * 1e9
    res["phase2_launch_ns"] = est[1] * 1e9
    return res


def kernel(**inputs):
    out, _ = _run(inputs)
    return out


# revision 36
# speedup vs baseline: 1.0424x; 1.0041x over previous
"""Trainium2 Bass kernel for nn_BHSDuelingDQN (gnn_message_passing).

Math notes (validated vs reference to fp32 precision):
  - The edge MLP input is ones(E,1), so every edge shares one theta [F,OUT]:
        theta = (relu(w1[0]+b1) @ w2 + b2).reshape(F, OUT)
  - edge_index values live in [0, N), so the gather/scatter-add only touches
    batch 0 of flat=[B*N,F].  With C[t,s] = #edges(src=s, tgt=t):
        agg(batch0) = C @ (x[0] @ theta)
    which turns the whole message passing into dense matmuls.

Sharding: phase 1 is node-sharded (each of 8 cores owns 32 of 256 nodes and
computes partial pre-activations of adv/v1 for all 1024 batches over its
4096 feature rows).  Phase 2 is batch-sharded (each core sums the 8 partials
for its 128 batches and runs the small value-head + dueling combine).  The
host only slices / concatenates / transposes arrays between phases.

Phase 1 conv trick: x is packed dense ([128 part] = 16 nodes x 8 feats) and
the per-node K=8 conv matmul uses a K=32 zero-padded stationary (root_w at
sub-offset 8j of a 32-row tile, zeros elsewhere) built on device, so no DMA
bytes are wasted on layout padding.

Layouts are transposed ([feature, batch]) throughout so no on-device
transposes are ever needed.
"""

import os
from contextlib import ExitStack

import numpy as np

import concourse.bacc as bacc
import concourse.bass as bass
import concourse.mybir as mybir
import concourse.tile as tile
from concourse import masks
from concourse.bass_utils import run_bass_kernel_spmd  # noqa: F401  (spmd entry)

F32 = mybir.dt.float32
BF16 = mybir.dt.bfloat16

B, N, F, E, OUT, NDIV, PER = 1024, 256, 8, 1024, 128, 64, 3
NADV = NDIV * PER            # 192
AV = NADV + 64               # 256 fused output cols of phase 1 (adv | v1)
M = 8                        # cores
NPC = N // M                 # 32 nodes per core

# hot-path matmul dtype mode: 'f32' (exact, 4x slower PE), 'f32r' (full rate,
# reduced-precision fp32 path), 'bf16' (half DMA traffic)
MODE = os.environ.get("BASS_KERNEL_MODE", "f32r")
NQ = int(os.environ.get("BASS_P2_NQ", "2"))   # phase-2 batch chunks
WARMUP = int(os.environ.get("BASS_WARMUP", "0"))  # dummy PE warmup matmuls

_build_cache = {}


def _np_bf16():
    import ml_dtypes

    return np.dtype(ml_dtypes.bfloat16)


def _np_store_dtype(mode):
    if mode == "bf16":
        return _np_bf16()
    return np.dtype(np.float32)


def _store_dt(mode):
    if mode == "bf16":
        return BF16
    if mode == "f32r":
        return mybir.dt.float32r  # fp32 bytes, PE full-rate reduced-precision
    return F32


# pa column layout: crit block first (needed before first conv), rest after.
PA_W1T, PA_B1T, PA_CB, PA_HB, PA_RW, PA_B2T = 0, 1, 2, 3, 5, 133
PA_CRIT = 141
PA_X0T, PA_X0TL, PA_W2 = 141, 397, 429
PA_COLS = 429 + 64 * 16  # w2 is [64, F*OUT=1024]
PB_COLS = 8 * N + 8 * NPC  # soh | tohl


def _build_phase1(mode, repeat=1):
    store_dt = _store_dt(mode)
    nc = bacc.Bacc("TRN2")

    pa_d = nc.dram_tensor("pa", [128, PA_COLS], F32, kind="ExternalInput")
    stw_d = nc.dram_tensor("stw", [128, 4 * OUT], store_dt, kind="ExternalInput")
    xd_d = nc.dram_tensor("xd", [128, 2, B], store_dt, kind="ExternalInput")
    wf_d = nc.dram_tensor("wf", [128, NPC, AV], store_dt, kind="ExternalInput")
    pb_d = nc.dram_tensor("pb", [128, PB_COLS], BF16, kind="ExternalInput")
    pt_d = nc.dram_tensor("pt", [AV, B], BF16, kind="ExternalOutput")

    with tile.TileContext(nc) as tc:
      for rep in range(repeat):
        with ExitStack() as ctx:
            const = ctx.enter_context(tc.tile_pool(name=f"const{rep}", bufs=1))

            pa_sb = const.tile([128, PA_COLS], F32)
            st_sb = const.tile([128, 4 * OUT], store_dt)
            xd_sb = const.tile([128, 2, B], store_dt)
            wf_sb = const.tile([128, NPC, AV], store_dt)
            pb_sb = const.tile([128, PB_COLS], BF16)

            # first-conv dependencies spread across queues so their HWDGE
            # setups overlap: stw+biases on scalar, x slab on sync
            nc.scalar.dma_start(out=st_sb, in_=stw_d[:])
            nc.sync.dma_start(out=xd_sb[:, 0, 512:B], in_=xd_d[:, 0, 512:B])
            nc.scalar.dma_start(out=pa_sb[:, 0:PA_CRIT], in_=pa_d[:, 0:PA_CRIT])
            # streaming weight loads in consumption order on sync;
            # h=1 sweep (batch 512:1024) runs first.
            nc.sync.dma_start(out=wf_sb[:, 0:8], in_=wf_d[:, 0:8])
            nc.sync.dma_start(out=xd_sb[:, 1, 512:B], in_=xd_d[:, 1, 512:B])
            nc.sync.dma_start(out=wf_sb[:, 8:16], in_=wf_d[:, 8:16])
            nc.sync.dma_start(out=xd_sb[:, :, 0:512], in_=xd_d[:, :, 0:512])
            nc.sync.dma_start(out=wf_sb[:, 16:24], in_=wf_d[:, 16:24])
            nc.sync.dma_start(out=wf_sb[:, 24:NPC], in_=wf_d[:, 24:NPC])
            # late consumers (phase 0b) on the otherwise-idle pool queue
            nc.gpsimd.dma_start(out=pb_sb, in_=pb_d[:])
            nc.gpsimd.dma_start(
                out=pa_sb[:, PA_CRIT:PA_COLS], in_=pa_d[:, PA_CRIT:PA_COLS]
            )

            def pa(col, ncols, nrows=128):
                return pa_sb[0:nrows, col : col + ncols]

            cb_ap = pa(PA_CB, 1)

            ident_sb = const.tile([128, 128], F32)
            masks.make_identity(nc, ident_sb)

            h_sb = const.tile([64, 1], F32)
            thT_sb = const.tile([128, F], F32)
            th_sb = const.tile([F, OUT], F32)
            x0th_sb = const.tile([128, 2, OUT], F32)
            ctl_sb = const.tile([128, 2, NPC], F32)
            feat0_sb = const.tile([128, NPC], store_dt)

            # --- phase 0a: theta and x0@theta (exact fp32) --------------
            with tc.tile_pool(name=f"p0a{rep}", bufs=1, space="PSUM") as p0a:
                # h = relu(w1 + b1) on DVE (no activation-table dep)
                nc.vector.tensor_scalar(
                    h_sb,
                    pa(PA_W1T, 1, 64),
                    pa(PA_B1T, 1, 64),
                    0.0,
                    mybir.AluOpType.add,
                    mybir.AluOpType.max,
                )
                # thetaT[oc, f] = w2[:, f*128+oc]^T @ h   (8 N=1 matmuls)
                thT_ps = p0a.tile([128, F], F32, tag="p0")
                for f in range(F):
                    nc.tensor.matmul(
                        thT_ps[:, f : f + 1],
                        pa(PA_W2 + f * OUT, OUT, 64),
                        h_sb,
                    )
                nc.vector.tensor_add(thT_sb, thT_ps, pa(PA_B2T, F))
                # theta [f, oc] via PE transpose
                th_ps = p0a.tile([F, OUT], F32, tag="p0")
                nc.tensor.transpose(th_ps, thT_sb[:, 0:F], ident_sb)
                nc.vector.tensor_copy(th_sb, th_ps)
                # x0 @ theta  [256 src, 128 oc]
                for s in range(2):
                    x0th_ps = p0a.tile(
                        [128, OUT], F32, name=f"x0th_ps{s}", tag="p0"
                    )
                    nc.tensor.matmul(
                        x0th_ps, pa(PA_X0T + s * 128, 128, F), th_sb
                    )
                    nc.vector.tensor_copy(x0th_sb[:, s, :], x0th_ps)

            # --- main pools --------------------------------------------
            acc_pool = ctx.enter_context(
                tc.tile_pool(name=f"accp{rep}", bufs=1, space="PSUM")
            )
            conv_pool = ctx.enter_context(
                tc.tile_pool(name=f"convp{rep}", bufs=3, space="PSUM")
            )
            feat_pool = ctx.enter_context(tc.tile_pool(name=f"featp{rep}", bufs=4))
            out_pool = ctx.enter_context(tc.tile_pool(name=f"outp{rep}", bufs=1))
            acc_ps = [acc_pool.tile([128, B], F32, name=f"acc{m}") for m in range(2)]
            ob0 = out_pool.tile([128, B], BF16, name="ob0")
            ob1 = out_pool.tile([128, B], BF16, name="ob1")

            pending = None

            def emit_node(n, h, parity):
                # conv for node n, half h; zero-padded K=32 stationary
                nonlocal pending
                g2, r = n // 16, n % 16
                a, j = r // 4, r % 4
                conv_ps = conv_pool.tile([128, 512], F32, name="conv_ps")
                nc.tensor.matmul(
                    conv_ps,
                    st_sb[32 * a : 32 * a + 32, OUT * j : OUT * (j + 1)],
                    xd_sb[32 * a : 32 * a + 32, g2, h * 512 : (h + 1) * 512],
                    tile_position=(32 * a, 0),
                )
                feat_sb = feat_pool.tile([128, 512], store_dt, name="feat_sb")
                if parity == 0:
                    nc.scalar.activation(
                        feat_sb,
                        conv_ps,
                        mybir.ActivationFunctionType.Relu,
                        bias=cb_ap,
                    )
                else:
                    nc.vector.tensor_scalar(
                        feat_sb,
                        conv_ps,
                        cb_ap,
                        0.0,
                        mybir.AluOpType.add,
                        mybir.AluOpType.max,
                    )
                if h == 0:
                    # batch 0 lives in column 0: overwrite this node's
                    # first column with the precomputed feat0 value
                    nc.vector.tensor_copy(
                        feat_sb[:, 0:1], feat0_sb[:, n : n + 1]
                    )
                flush_acc()
                pending = (n, h, feat_sb)

            def flush_acc():
                nonlocal pending
                if pending is not None:
                    pn, ph, pf = pending
                    for m in range(2):
                        nc.tensor.matmul(
                            acc_ps[m][:, ph * 512 : (ph + 1) * 512],
                            wf_sb[:, pn, m * 128 : (m + 1) * 128],
                            pf,
                            start=(pn == 0),
                            stop=(pn == NPC - 1),
                        )
                    pending = None

            def drain_half(h):
                sl = slice(h * 512, (h + 1) * 512)
                nc.vector.tensor_copy(ob0[:, sl], acc_ps[0][:, sl])
                nc.scalar.activation(
                    ob1[:, sl],
                    acc_ps[1][:, sl],
                    mybir.ActivationFunctionType.Copy,
                )
                nc.sync.dma_start(out=pt_d[0:128, sl], in_=ob0[:, sl])
                nc.scalar.dma_start(out=pt_d[128:AV, sl], in_=ob1[:, sl])

            for n in range(NPC):
                emit_node(n, 1, n % 2)
            flush_acc()
            drain_half(1)

            # --- phase 0b: local count matrix and agg (needs pb) --------
            p0b = conv_pool
            for s in range(2):
                ctl_ps = p0b.tile(
                    [128, NPC], F32, name=f"ctl_ps{s}", tag="conv_ps"
                )
                for k in range(8):
                    nc.tensor.matmul(
                        ctl_ps,
                        pb_sb[:, k * N + s * 128 : k * N + (s + 1) * 128],
                        pb_sb[:, 8 * N + k * NPC : 8 * N + (k + 1) * NPC],
                        start=(k == 0),
                        stop=(k == 7),
                    )
                nc.vector.tensor_copy(ctl_sb[:, s, :], ctl_ps)
            agg_ps = p0b.tile([128, NPC], F32, tag="conv_ps")
            for s in range(2):
                nc.tensor.matmul(
                    agg_ps,
                    x0th_sb[:, s, :],
                    ctl_sb[:, s, :],
                    start=(s == 0),
                    stop=False,
                )
            # + conv of batch 0 for the local nodes -> agg_ps holds the
            # full pre-activation of batch 0 (minus conv_b)
            nc.tensor.matmul(
                agg_ps,
                pa(PA_RW, OUT, F),
                pa(PA_X0TL, NPC, F),
                start=False,
                stop=True,
            )
            # feat0[oc, n] = relu(conv0 + agg + conv_b): the correct
            # batch-0 feature column for every local node
            nc.scalar.activation(
                feat0_sb,
                agg_ps,
                mybir.ActivationFunctionType.Relu,
                bias=cb_ap,
            )

            for n in range(NPC):
                emit_node(n, 0, (n + 1) % 2)
            flush_acc()
            drain_half(0)

    nc.finalize()
    return nc


BT = B // M      # 128 batches per core
BH = BT // NQ    # batch chunk per pipeline stage

# pk (merged bf16 consts) column layout
PK_MGA, PK_MGB, PK_EM0, PK_EM1, PK_V2WZ, PK_V3 = 0, 64, 128, 256, 320, 384
PK_COLS = 448
P2_WARM = 10


def _build_phase2(repeat=1):
    nc = bacc.Bacc("TRN2")

    parts_d = nc.dram_tensor(
        "parts", [128, NQ, M, 2, BH], BF16, kind="ExternalInput"
    )
    pk_d = nc.dram_tensor("pk", [128, PK_COLS], BF16, kind="ExternalInput")
    pf_d = nc.dram_tensor("pf", [128, 3], F32, kind="ExternalInput")
    ot_d = nc.dram_tensor("ot", [128, NQ, 2, BH], F32, kind="ExternalOutput")

    with tile.TileContext(nc) as tc:
      for rep in range(repeat):
        with ExitStack() as ctx:
            const = ctx.enter_context(tc.tile_pool(name=f"c2_{rep}", bufs=1))

            pp_sb = const.tile([128, NQ, M, 2, BH], BF16, name="pp_sb")
            for q in range(NQ):
                nc.sync.dma_start(out=pp_sb[:, q], in_=parts_d[:, q])
            pk_sb = const.tile([128, PK_COLS], BF16)
            nc.scalar.dma_start(out=pk_sb, in_=pk_d[:])
            pf_sb = const.tile([128, 3], F32)
            nc.gpsimd.dma_start(out=pf_sb, in_=pf_d[:])

            # device-built identity (bf16) for the partial-sum matmuls;
            # ready long before any DMA lands
            ident_sb = const.tile([128, 128], BF16)
            masks.make_identity(nc, ident_sb)

            work = ctx.enter_context(tc.tile_pool(name=f"work{rep}", bufs=2))
            psum = ctx.enter_context(
                tc.tile_pool(name=f"psum{rep}", bufs=2, space="PSUM")
            )
            warmp = ctx.enter_context(
                tc.tile_pool(name=f"warmp{rep}", bufs=1, space="PSUM")
            )
            warm_ps = warmp.tile([128, 128], F32, name="warm_ps")
            for w in range(P2_WARM):
                nc.tensor.matmul(warm_ps, ident_sb, ident_sb)
            ot_sb = const.tile([128, NQ, 2, BH], F32, name="ot_sb")
            # rows 64:128 of the packed second half are never written;
            # zero once so the packed DMA reads defined bytes
            nc.gpsimd.memset(ot_sb[64:128, :, 1], 0.0)

            for hb in range(NQ):
                # sum the 8 partials on the PE (identity-stationary
                # accumulation into PSUM), both feature blocks at once
                s_ps = psum.tile([128, 2, BH], F32, name="s_ps")
                for c in range(M):
                    nc.tensor.matmul(
                        s_ps,
                        ident_sb,
                        pp_sb[:, hb, c],
                        start=(c == 0),
                        stop=(c == M - 1),
                    )
                # ar = relu(s + head_bias), per feature block
                ar = work.tile([128, 2, BH], BF16, name="ar")
                for fb in range(2):
                    nc.vector.tensor_scalar(
                        ar[:, fb],
                        s_ps[:, fb],
                        pf_sb[:, fb : fb + 1],
                        0.0,
                        mybir.AluOpType.add,
                        mybir.AluOpType.max,
                    )
                # small PSUM scratch: v2 | vm | dx0 | dx1 packed in one bank
                sm_ps = psum.tile([128, 4, BH], F32, name="sm_ps")
                v2_ps = sm_ps[0:64, 0]
                vm_ps = sm_ps[0:64, 1]
                dx0_ps = sm_ps[:, 2]
                dx1_ps = sm_ps[0:64, 3]
                # v2 = relu(v2w^T v1 + v2b); v1 sits at rows 64:128 of fb1
                nc.tensor.matmul(
                    v2_ps, pk_sb[:, PK_V2WZ : PK_V2WZ + 64], ar[:, 1]
                )
                v2r = work.tile([65, BH], BF16, name="v2r")
                nc.vector.tensor_scalar(
                    v2r[0:64],
                    v2_ps,
                    pf_sb[0:64, 2:3],
                    0.0,
                    mybir.AluOpType.add,
                    mybir.AluOpType.max,
                )
                nc.vector.memset(v2r[64:65], 1.0)

                # vm = v3w^T v2 + v3b - (1/3) group-sums(adv)
                nc.tensor.matmul(
                    vm_ps,
                    pk_sb[0:65, PK_V3 : PK_V3 + 64],
                    v2r,
                    start=True,
                    stop=False,
                )
                nc.tensor.matmul(
                    vm_ps,
                    pk_sb[:, PK_MGA : PK_MGA + 64],
                    ar[:, 0],
                    start=False,
                    stop=False,
                )
                nc.tensor.matmul(
                    vm_ps,
                    pk_sb[:, PK_MGB : PK_MGB + 64],
                    ar[:, 1],
                    start=False,
                    stop=True,
                )
                d1 = work.tile([64, BH], BF16, name="d1")
                nc.vector.tensor_copy(d1, vm_ps)

                # expand to (d,p) rows and add adv
                nc.tensor.matmul(
                    dx0_ps, pk_sb[0:64, PK_EM0 : PK_EM0 + 128], d1
                )
                nc.tensor.matmul(
                    dx1_ps, pk_sb[0:64, PK_EM1 : PK_EM1 + 64], d1
                )
                nc.vector.tensor_add(ot_sb[:, hb, 0], ar[:, 0], dx0_ps)
                nc.vector.tensor_add(ot_sb[0:64, hb, 1], ar[0:64, 1], dx1_ps)
                # one packed output DMA per chunk, on otherwise-idle queues
                q = nc.scalar if hb == 0 else nc.sync
                q.dma_start(out=ot_d[:, hb], in_=ot_sb[:, hb])

    nc.finalize()
    return nc


def _get_programs(mode, repeat=1):
    key = (mode, repeat, NQ)
    if key not in _build_cache:
        _build_cache[key] = (_build_phase1(mode, repeat), _build_phase2(repeat))
    return _build_cache[key]


def _prep_phase1_inputs(inputs, mode):
    sd = _np_store_dtype(mode)
    bf = _np_bf16()
    x = np.ascontiguousarray(np.asarray(inputs["x"], np.float32))
    ei = np.asarray(inputs["edge_index"]).astype(np.int64)
    w1 = np.asarray(inputs["w1"], np.float32)
    b1 = np.asarray(inputs["b1"], np.float32)
    w2 = np.asarray(inputs["w2"], np.float32)
    b2 = np.asarray(inputs["b2"], np.float32)
    root_w = np.asarray(inputs["root_w"], np.float32)
    conv_b = np.asarray(inputs["conv_b"], np.float32)
    adv_w = np.asarray(inputs["adv_w"], np.float32)
    v1w = np.asarray(inputs["v1w"], np.float32)

    src_i, tgt_i = ei[0], ei[1]
    wfull = np.concatenate([adv_w, v1w], axis=1)  # [32768, 256]

    pa = np.zeros((128, PA_COLS), np.float32)
    pa[0:64, PA_W1T] = w1.reshape(64)
    pa[0:64, PA_B1T] = b1
    pa[:, PA_CB] = conv_b
    pa[0:F, PA_RW : PA_RW + OUT] = root_w
    pa[:, PA_B2T : PA_B2T + F] = b2.reshape(F, OUT).T
    pa[0:F, PA_X0T : PA_X0T + N] = x[0].T
    pa[0:64, PA_W2 : PA_W2 + F * OUT] = w2

    soh = (src_i[:, None] == np.arange(N)[None, :]).astype(np.float32)
    soh = soh.reshape(8, 128, N).transpose(1, 0, 2).reshape(128, 8 * N)

    # zero-padded conv stationary: stw[32a+8j+f, 128j+oc] = root_w[f, oc]
    stw = np.zeros((128, 4 * OUT), np.float32)
    for a in range(4):
        for j in range(4):
            stw[32 * a + 8 * j : 32 * a + 8 * j + F, OUT * j : OUT * (j + 1)] = (
                root_w
            )
    stw = stw.astype(sd)

    in_maps = []
    for c in range(M):
        pac = pa.copy()
        pac[0:F, PA_X0TL : PA_X0TL + NPC] = x[0, NPC * c : NPC * (c + 1), :].T
        pb = np.zeros((128, PB_COLS), np.float32)
        pb[:, 0 : 8 * N] = soh
        tohl = (tgt_i[:, None] == (NPC * c + np.arange(NPC))[None, :]).astype(
            np.float32
        )
        pb[:, 8 * N :] = tohl.reshape(8, 128, NPC).transpose(1, 0, 2).reshape(
            128, 8 * NPC
        )

        # dense x pack: node n -> rows 32a+8j (a=(n%16)//4, j=n%4), slab n//16
        xc = x[:, NPC * c : NPC * (c + 1), :]           # [B, 32, 8]
        xr = xc.transpose(1, 2, 0)                      # [32, 8, B]
        xd = (
            xr.reshape(2, 4, 4, F, B)
            .transpose(1, 2, 3, 0, 4)
            .reshape(128, 2, B)
        )
        wfc = (
            wfull[4096 * c : 4096 * (c + 1)]            # [4096, 256]
            .reshape(NPC, 128, AV)
            .transpose(1, 0, 2)                         # [128, 32, 256]
        )
        in_maps.append(
            {
                "pa": pac,
                "stw": stw,
                "xd": xd.astype(sd),
                "wf": np.ascontiguousarray(wfc).astype(sd),
                "pb": pb.astype(bf),
            }
        )
    return in_maps


def _prep_phase2_inputs(inputs, pts):
    bf = _np_bf16()
    adv_b = np.asarray(inputs["adv_b"], np.float32)
    v1b = np.asarray(inputs["v1b"], np.float32)
    v2w = np.asarray(inputs["v2w"], np.float32)
    v2b = np.asarray(inputs["v2b"], np.float32)
    v3w = np.asarray(inputs["v3w"], np.float32)
    v3b = np.asarray(inputs["v3b"], np.float32)

    dp = np.arange(NADV)
    mg = np.zeros((NADV, NDIV), np.float32)
    mg[dp, dp // PER] = -1.0 / PER           # negated group-mean matrix
    em = np.zeros((NDIV, NADV), np.float32)  # expand d -> (d,p)
    em[dp // PER, dp] = 1.0
    pk = np.zeros((128, PK_COLS), np.float32)
    pk[:, PK_MGA : PK_MGA + 64] = mg[0:128]
    pk[0:64, PK_MGB : PK_MGB + 64] = mg[128:NADV]
    pk[0:64, PK_EM0 : PK_EM0 + 128] = em[:, 0:128]
    pk[0:64, PK_EM1 : PK_EM1 + 64] = em[:, 128:NADV]
    pk[64:128, PK_V2WZ : PK_V2WZ + 64] = v2w
    pk[0:64, PK_V3 : PK_V3 + 64] = v3w
    pk[64, PK_V3 : PK_V3 + 64] = v3b
    pf = np.zeros((128, 3), np.float32)
    pf[:, 0] = adv_b[0:128]
    pf[0:64, 1] = adv_b[128:NADV]
    pf[64:128, 1] = v1b
    pf[0:64, 2] = v2b

    pts_f = [np.asarray(p, np.float32) for p in pts]    # [256, 1024] each
    in_maps = []
    for c in range(M):
        parts = np.zeros((128, NQ, M, 2, BH), np.float32)
        for cc in range(M):
            blk = pts_f[cc][:, BT * c : BT * (c + 1)]   # [256, 128]
            parts[:, :, cc, 0, :] = blk[0:128].reshape(128, NQ, BH)
            parts[:, :, cc, 1, :] = blk[128:256].reshape(128, NQ, BH)
        in_maps.append(
            {
                "parts": parts.astype(bf),
                "pk": pk.astype(bf),
                "pf": pf,
            }
        )
    return in_maps


class _Runner:
    """Cached PJRT executor for one Bass program across the 8 cores.

    Mirrors bass2jax.run_bass_via_pjrt but keeps the jitted callable so
    repeat calls don't re-trace/re-lower, enabling benchmarking.
    """

    def __init__(self, nc):
        import jax
        from jax.sharding import Mesh, PartitionSpec, NamedSharding
        from jax.experimental.shard_map import shard_map
        from concourse import bass2jax

        bass2jax.install_neuronx_cc_hook()
        self.jax = jax
        self.nc = nc
        partition_name = (
            nc.partition_id_tensor.name if nc.partition_id_tensor else None
        )
        in_names, out_names, out_avals, zero_shapes = [], [], [], []
        for alloc in nc.m.functions[0].allocations:
            if not isinstance(alloc, mybir.MemoryLocationSet):
                continue
            name = alloc.memorylocations[0].name
            if alloc.kind == "ExternalInput":
                if name != partition_name:
                    in_names.append(name)
            elif alloc.kind == "ExternalOutput":
                shape = tuple(alloc.tensor_shape)
                dtype = mybir.dt.np(alloc.dtype)
                out_names.append(name)
                out_avals.append(jax.core.ShapedArray(shape, dtype))
                zero_shapes.append((shape, dtype))
        self.in_names, self.out_names = in_names, out_names
        self.out_avals, self.zero_shapes = out_avals, zero_shapes
        n_params, n_outs = len(in_names), len(out_names)
        self.n_params = n_params

        bind_names = in_names + out_names
        if partition_name is not None:
            bind_names = bind_names + [partition_name]

        def _body(*args):
            operands = list(args)
            if partition_name is not None:
                operands.append(bass2jax.partition_id_tensor())
            outs = bass2jax._bass_exec_p.bind(
                *operands,
                out_avals=tuple(out_avals),
                in_names=tuple(bind_names),
                out_names=tuple(out_names),
                lowering_input_output_aliases=(),
                sim_require_finite=True,
                sim_require_nnan=True,
                nc=nc,
            )
            return tuple(outs)

        devices = jax.devices()[:M]
        self.mesh = Mesh(np.asarray(devices), ("core",))
        spec = PartitionSpec("core")
        self.sharding = NamedSharding(self.mesh, spec)
        donate = tuple(range(n_params, n_params + n_outs))
        self.fn = jax.jit(
            shard_map(
                _body,
                mesh=self.mesh,
                in_specs=(spec,) * (n_params + n_outs),
                out_specs=(spec,) * n_outs,
                check_rep=False,
            ),
            donate_argnums=donate,
            keep_unused=True,
        )

    def _concat_inputs(self, in_maps):
        return [
            np.concatenate([np.asarray(m[name]) for m in in_maps], axis=0)
            for name in self.in_names
        ]

    def _zeros(self):
        return [np.zeros((M * s[0], *s[1:]), d) for s, d in self.zero_shapes]

    def _split(self, out_arrs):
        res = []
        for c in range(M):
            res.append(
                {
                    name: np.asarray(out_arrs[i]).reshape(M, *self.out_avals[i].shape)[c]
                    for i, name in enumerate(self.out_names)
                }
            )
        return res

    def run(self, in_maps):
        out_arrs = self.fn(*self._concat_inputs(in_maps), *self._zeros())
        return self._split(out_arrs)

    def bench(self, in_maps, iters=20):
        import time

        jax = self.jax
        dev_in = [
            jax.device_put(a, self.sharding) for a in self._concat_inputs(in_maps)
        ]
        times = []
        out_arrs = None
        for _ in range(iters):
            zeros = [jax.device_put(z, self.sharding) for z in self._zeros()]
            jax.block_until_ready(zeros)
            t0 = time.perf_counter()
            out_arrs = self.fn(*dev_in, *zeros)
            jax.block_until_ready(out_arrs)
            times.append(time.perf_counter() - t0)
        return self._split(out_arrs), times


_runner_cache = {}


def _get_runner(nc, key):
    if key not in _runner_cache:
        _runner_cache[key] = _Runner(nc)
    return _runner_cache[key]


def _run_sim(nc, in_maps):
    from concourse.bass_interp import CoreSim

    outs = []
    for im in in_maps:
        sim = CoreSim(nc)
        for k, v in im.items():
            sim.tensor(k)[:] = v
        sim.simulate()
        out_name = "pt" if "xd" in im else "ot"
        outs.append({out_name: np.array(sim.tensor(out_name))})
    return outs


def _run(inputs, mode=None, trace=False, backend="hw", bench_iters=0):
    mode = mode or MODE
    nc1, nc2 = _get_programs(mode)
    info = {}

    in_maps1 = _prep_phase1_inputs(inputs, mode)
    if backend == "sim":
        res1 = _run_sim(nc1, in_maps1)
    else:
        runner1 = _get_runner(nc1, ("p1", mode))
        if bench_iters:
            res1, times = runner1.bench(in_maps1, bench_iters)
            info["phase1_ns"] = int(min(times) * 1e9)
            info["phase1_mean_ns"] = float(np.mean(times) * 1e9)
        else:
            res1 = runner1.run(in_maps1)
    pts = [res1[c]["pt"] for c in range(M)]

    in_maps2 = _prep_phase2_inputs(inputs, pts)
    if backend == "sim":
        res2 = _run_sim(nc2, in_maps2)
    else:
        runner2 = _get_runner(nc2, ("p2", NQ))
        if bench_iters:
            res2, times = runner2.bench(in_maps2, bench_iters)
            info["phase2_ns"] = int(min(times) * 1e9)
            info["phase2_mean_ns"] = float(np.mean(times) * 1e9)
        else:
            res2 = runner2.run(in_maps2)

    out = np.empty((B, NDIV, PER), np.float32)
    for c in range(M):
        ot = np.asarray(res2[c]["ot"], np.float32)  # [128, NQ, 2, BH]
        full = np.concatenate(
            [ot[:, :, 0, :], ot[0:64, :, 1, :]], axis=0
        ).reshape(NADV, BT)
        out[BT * c : BT * (c + 1)] = full.T.reshape(BT, NDIV, PER)
    return out, info


def _p25(ts):
    ts = sorted(ts)
    return ts[max(0, len(ts) // 4)]


def bench_hw(inputs, mode=None, big_rep=9, iters=12):
    """Differential HW timing: (T(R) - T(1)) / (R - 1) cancels the axon
    launch overhead and measures the true per-pass device time.  Uses the
    25th percentile (the min is occasionally glitchy on the relay)."""
    mode = mode or MODE
    in_maps1 = _prep_phase1_inputs(inputs, mode)
    res = {}
    est = {}
    for r in (1, big_rep):
        nc1, _ = _get_programs(mode, r)
        runner = _get_runner(nc1, ("p1", mode, r))
        out1, times = runner.bench(in_maps1, iters)
        est[r] = _p25(times)
    res["phase1_ns"] = (est[big_rep] - est[1]) / (big_rep - 1) * 1e9
    res["phase1_launch_ns"] = est[1] * 1e9

    pts = [o["pt"] for o in out1]
    in_maps2 = _prep_phase2_inputs(inputs, pts)
    for r in (1, big_rep):
        _, nc2 = _get_programs(mode, r)
        runner = _get_runner(nc2, ("p2", NQ, r))
        _, times = runner.bench(in_maps2, iters)
        est[r] = _p25(times)
    res["phase2_ns"] = (est[big_rep] - est[1]) / (big_rep - 1) * 1e9
    res["phase2_launch_ns"] = est[1] * 1e9
    return res


def kernel(**inputs):
    out, _ = _run(inputs)
    return out
